# revision 14
# baseline (speedup 1.0000x reference)
"""Binary-weight 3x3 conv (BinaryConv2d) Trainium2 Bass kernel.

Reference computation (x[32,256,56,56] f32, w[256,256,3,3] f32, b[256] f32):
    out = conv2d(x, sign(w), pad=1) + sign(b)[None,:,None,None]

Strategy (v2 — F(4,3) Winograd along H, direct 3-tap along W):
  - Data-parallel over batch: 8 cores x 4 images each. No collectives.
  - PE does 6(k-plane) x 3(kx) x 2(ki) matmuls per 7-band chunk instead of
    9 x 2 direct taps: 4.5 MACs/output vs 9 (2x fewer PE row-cycles; the
    direct kernel is PE-bound at 451.6us/iter locally).
  - The local backend charges ~0.6-1.5us fixed cost per vector-engine op,
    so transforms are organized as few, wide "mega-ops": strided
    multi-component access patterns compute up to 4 subexpressions per
    instruction. Forward transform: 10 vector ops per (ki, image) via a
    packed subexpression tile; V-plane order is permuted (and U2 negated)
    so combine steps pair into affine 2-component ops.
  - Work split: GPSIMD runs the forward transform (SBUF-only ops), DVE
    runs the inverse (PSUM reads are DVE-only), ACT only does the
    f32->fp16 padded-image copy, bias rides the PE as a K=1 ones-matmul
    accumulated into the M1-plane psum group (A^T column 1 is all-ones).
  - Everything fp16 (exact enough: rel err ~5e-3 << 2e-2 gate), output
    stored fp16 and upcast on host.
"""

from contextlib import ExitStack

import numpy as np

import concourse.bacc as bacc
import concourse.bass as bass
import concourse.tile as tile
import concourse.mybir as mybir
from concourse import masks
from concourse.bass_utils import run_bass_kernel_spmd

F32 = mybir.dt.float32
F16 = mybir.dt.float16

N_CORES = 8
B, C, H, W = 32, 256, 56, 56
O = 256
KH = KW = 3
BPC = B // N_CORES  # images per core
KI = C // 128       # input-channel chunks
OC = O // 128       # output-channel chunks

M = 4               # winograd output rows per tile: F(4,3)
T = M + 2           # transformed planes
NT = H // M         # tile-row bands per image (14)
NCH = 2             # band chunks per image for matmul/psum (7 bands each)

AL = mybir.AluOpType

# V-plane position -> original winograd k index (U2 carries a flipped sign:
# position 5 holds -V2, compensated by negating U_2 and swapping P/Q roles).
#   pos: [V0, V5, V3, V4, V1, -V2]
POS_K = [0, 5, 3, 4, 1, 2]


def build_program(bpc=BPC, h=H, w=W, repeat=1):
    """Build the per-core Bass program. Returns compiled nc."""
    assert h % M == 0
    nt = h // M
    cb = nt // NCH          # bands per chunk (7)
    fd = cb * w             # matmul free size (392)
    pw = w + 2              # V width with conv column pads

    nc = bacc.Bacc("TRN2", target_bir_lowering=False, debug=False,
                   num_devices=N_CORES)
    x_d = nc.dram_tensor("x", [bpc, C, h, w], F32, kind="ExternalInput").ap()
    w_d = nc.dram_tensor("weight", [O, C, KH, KW], F32,
                         kind="ExternalInput").ap()
    b_d = nc.dram_tensor("bias", [O], F32, kind="ExternalInput").ap()
    o_d = nc.dram_tensor("out", [bpc, O, h, w], F16, kind="ExternalOutput").ap()

    with tile.TileContext(nc) as tc, ExitStack() as ctx:
        const = ctx.enter_context(tc.tile_pool(name="const", bufs=1))
        xstg_p = ctx.enter_context(tc.tile_pool(name="xstg", bufs=2))
        xpad_p = ctx.enter_context(tc.tile_pool(name="xpad", bufs=2))
        v_p = ctx.enter_context(tc.tile_pool(name="vp", bufs=2))
        s_p = ctx.enter_context(tc.tile_pool(name="sp", bufs=2))
        out_p = ctx.enter_context(tc.tile_pool(name="outp", bufs=4))

        # ---- constants ----
        identity = const.tile([128, 128], F16)
        masks.make_identity(nc, identity[:])

        ones_row = const.tile([1, 512], F16)
        nc.gpsimd.memset(ones_row[:], 1.0)
        b_raw = const.tile([1, O], F32)
        nc.sync.dma_start(out=b_raw[:], in_=b_d.rearrange("(a b) -> a b", a=1))
        b_row = const.tile([1, O], F16)
        nc.scalar.sign(b_row[:], b_raw[:])

        # ---- weights: load, binarize, transpose, G-combine along ky ----
        # lhsT_U[:, idxu, :] = U'_p[kx, ki, oc] with p the V position order;
        #   idxu = ((p*KW + kx)*KI + ki)*OC + oc
        # U'_p = G-combo for k=POS_K[p], negated for p=5.
        NB = KW * KI * OC  # tiles per k block (12)

        def idx_raw(ky, kx, ki, oc):
            return ((ky * KW + kx) * KI + ki) * OC + oc

        lhsT_U = const.tile([128, T * NB, 128], F16)

        wstg_ctx = ExitStack()
        wstg_p = wstg_ctx.enter_context(tc.tile_pool(name="wstg", bufs=2))
        tpsum_p = wstg_ctx.enter_context(
            tc.tile_pool(name="tpsum", bufs=2, space=bass.MemorySpace.PSUM))
        lhsT_raw = wstg_p.tile([128, KH * KW * KI * OC, 128], F16, tag="raw",
                               bufs=1)
        for ki in range(KI):
            for oc in range(OC):
                wstg = wstg_p.tile([128, 128, KH, KW], F32, tag="wstg")
                nc.sync.dma_start(
                    out=wstg[:],
                    in_=w_d[oc * 128:(oc + 1) * 128,
                            ki * 128:(ki + 1) * 128, :, :])
                wbin = wstg_p.tile([128, 128, KH, KW], F16, tag="wbin",
                                   bufs=1)
                nc.scalar.sign(wbin[:], wstg[:])
                for ky in range(KH):
                    for kx in range(KW):
                        tp = tpsum_p.tile([128, 128], F16)
                        nc.tensor.transpose(tp[:], wbin[:, :, ky, kx],
                                            identity[:])
                        nc.vector.tensor_copy(
                            lhsT_raw[:, idx_raw(ky, kx, ki, oc), :], tp[:])

        # G rows (k): U0=g0/4, U1=-(g0+g1+g2)/6, U2=(g1-g0-g2)/6,
        #             U3=(g0+2g1+4g2)/24, U4=(g0-2g1+4g2)/24, U5=g2
        def rawb(ky):
            return lhsT_raw[:, ky * NB:(ky + 1) * NB, :]

        def ub(k_pos):
            return lhsT_U[:, k_pos * NB:(k_pos + 1) * NB, :]

        # position mapping: ub(pos) gets U_{POS_K[pos]} (neg for pos 5)
        UPOS = {k: p for p, k in enumerate(POS_K)}
        g0, g1, g2 = rawb(0), rawb(1), rawb(2)
        wt1 = wstg_p.tile([128, NB, 128], F16, tag="wt1", bufs=1)
        wt2 = wstg_p.tile([128, NB, 128], F16, tag="wt2", bufs=1)
        nc.vector.tensor_scalar_mul(ub(UPOS[0]), g0, 0.25)
        nc.vector.tensor_copy(ub(UPOS[5]), g2)
        nc.vector.tensor_add(wt1[:], g0, g2)
        nc.vector.tensor_add(wt2[:], wt1[:], g1)
        nc.vector.tensor_scalar_mul(ub(UPOS[1]), wt2[:], -1.0 / 6.0)
        nc.vector.tensor_sub(wt2[:], g1, wt1[:])
        # +U2 here: position 5 stores -V2 data, so M_pos5 = U2 * (-V2) = -M2
        nc.vector.tensor_scalar_mul(ub(UPOS[2]), wt2[:], 1.0 / 6.0)
        nc.vector.tensor_add(wt1[:], g1, g1)
        nc.vector.tensor_add(wt2[:], wt1[:], g0)      # g0+2g1
        nc.vector.tensor_add(wt1[:], g2, g2)
        nc.vector.tensor_add(wt1[:], wt1[:], wt1[:])  # 4g2
        nc.vector.tensor_add(wt2[:], wt2[:], wt1[:])  # g0+2g1+4g2
        nc.vector.tensor_scalar_mul(ub(UPOS[3]), wt2[:], 1.0 / 24.0)
        nc.vector.tensor_sub(wt2[:], g0, g1)
        nc.vector.tensor_sub(wt2[:], wt2[:], g1)      # g0-2g1
        nc.vector.tensor_add(wt2[:], wt2[:], wt1[:])  # g0-2g1+4g2
        nc.vector.tensor_scalar_mul(ub(UPOS[4]), wt2[:], 1.0 / 24.0)
        wstg_ctx.close()

        psum_p = ctx.enter_context(
            tc.tile_pool(name="psum", bufs=8, space=bass.MemorySpace.PSUM))

        # ---- main loop over images ----
        for _rep in range(repeat):
            for n in range(bpc):
                V = {}
                for ki in range(KI):
                    # f32 row-padded staging (rows 1..56 data, 0 and 57 zero)
                    xfp = xstg_p.tile([128, h + 2, w], F32, tag="xfp")
                    hh = h // 2
                    nc.sync.dma_start(
                        out=xfp[:, 1:1 + hh, :],
                        in_=x_d[n, ki * 128:(ki + 1) * 128, :hh, :])
                    nc.sync.dma_start(
                        out=xfp[:, 1 + hh:1 + h, :],
                        in_=x_d[n, ki * 128:(ki + 1) * 128, hh:, :])
                    nc.gpsimd.memset(xfp[:, 0, :], 0.0)
                    nc.gpsimd.memset(xfp[:, h + 1, :], 0.0)
                    # fp16 copy (ACT), 60 rows so the q=4 band view divides
                    xp = xpad_p.tile([128, h + 4, w], F16, tag=f"xp{ki}")
                    nc.scalar.copy(xp[:, 0:h + 2, :], xfp[:])

                    # subexpression mega-ops.
                    # S components: 0:s7=d2-d0 1:s6=d3-d1 2:s5=d4-d2
                    #   3:s8=d5-d3 4:s2=d1+d2 5:s1=d3+d4 6:s3=d1-d2
                    #   7:s4'=d3-d4     (d_j = padded row 4r'+j)
                    st = s_p.tile([128, 8, nt, w], F16, tag="st")
                    hi = xp[:, 2:2 + 4 * nt, :].rearrange(
                        "p (r q) c -> p q r c", q=4)
                    lo = xp[:, 0:4 * nt, :].rearrange(
                        "p (r q) c -> p q r c", q=4)
                    nc.gpsimd.tensor_sub(st[:, 0:4], hi, lo)
                    odd = xp[:, 1:1 + 4 * nt, :].rearrange(
                        "p (r b t) c -> p t b r c", b=2, t=2)
                    nc.gpsimd.tensor_add(st[:, 4:6], odd[:, 0], odd[:, 1])
                    nc.gpsimd.tensor_sub(st[:, 6:8], odd[:, 0], odd[:, 1])

                    # scales via add-chains (GPS tensor_scalar is slow):
                    # tt: 0:4*s7 1:4*s6 2:4*s2 3:4*s3 4:2*s6
                    tt = s_p.tile([128, 5, nt, w], F16, tag="tt")
                    stv = st[:].rearrange("p s r c -> p s (r c)")
                    ttv = tt[:].rearrange("p s r c -> p s (r c)")
                    nc.gpsimd.tensor_add(ttv[:, 4], stv[:, 1], stv[:, 1])
                    nc.gpsimd.tensor_add(tt[:, 0:2], st[:, 0:2], st[:, 0:2])
                    nc.gpsimd.tensor_add(tt[:, 0:2], tt[:, 0:2], tt[:, 0:2])
                    s23 = st[:].rearrange("p (a s) r c -> p s a r c", s=2)
                    nc.gpsimd.tensor_add(tt[:, 2:4], s23[:, 0, 2:4],
                                         s23[:, 0, 2:4])
                    nc.gpsimd.tensor_add(tt[:, 2:4], tt[:, 2:4], tt[:, 2:4])

                    # V combines into position-ordered planes, cols 1..56
                    vt = v_p.tile([128, T, nt, pw], F16, tag=f"V{ki}")
                    nc.gpsimd.memset(vt[:, :, :, 0], 0.0)
                    nc.gpsimd.memset(vt[:, :, :, pw - 1], 0.0)
                    vin = vt[:, :, :, 1:w + 1]
                    # V combines on DVE to balance against GPSIMD's
                    # subexpression/scale load
                    # pos0 = V0 = s5-4s7 ; pos1 = V5 = s8-4s6
                    nc.vector.tensor_sub(vin[:, 0:2], st[:, 2:4], tt[:, 0:2])
                    # pos2 = V3 = s5+2s6 ; pos3 = V4 = s5-2s6
                    nc.vector.tensor_add(vin[:, 2], st[:, 2], tt[:, 4])
                    nc.vector.tensor_sub(vin[:, 3], st[:, 2], tt[:, 4])
                    # pos4 = V1 = s1-4s2 ; pos5 = -V2 = s4'-4s3
                    s17 = st[:].rearrange("p (a s) r c -> p s a r c", s=2)
                    nc.vector.tensor_sub(vin[:, 4:6], s17[:, 1, 2:4],
                                         tt[:, 2:4])
                    V[ki] = vt

                for oc in range(OC):
                    for ch in range(NCH):
                        r0 = ch * cb
                        ms = []
                        for p in range(T):
                            ps = psum_p.tile([128, cb, w], F32)
                            mm = 0
                            if p == 4:  # M1 group opens with the bias matmul
                                nc.tensor.matmul(
                                    ps[:],
                                    b_row[:, oc * 128:(oc + 1) * 128],
                                    ones_row[:, :fd].rearrange(
                                        "a (r c) -> a r c", c=w),
                                    start=True, stop=False)
                                mm = 1
                            for ki in range(KI):
                                for kx in range(KW):
                                    nc.tensor.matmul(
                                        ps[:],
                                        lhsT_U[:, ((p * KW + kx) * KI + ki)
                                               * OC + oc, :],
                                        V[ki][:, p, r0:r0 + cb, kx:kx + w],
                                        start=(mm == 0),
                                        stop=(mm == (KI * KW
                                                     + (1 if p == 4 else 0)
                                                     - 1)))
                                    mm += 1
                            ms.append(ps)

                        # inverse A^T on DVE (psum planes in position order:
                        # p0=M0 p1=M5 p2=M3 p3=M4 p4=M1(+bias) p5=-M2)
                        ob = out_p.tile([128, M * cb, w], F16, tag="ob")
                        obv = ob[:].rearrange("p (r q) c -> p q r c", q=M)
                        iP = s_p.tile([128, cb, w], F16, tag="iP")
                        iQ = s_p.tile([128, cb, w], F16, tag="iQ")
                        iR = s_p.tile([128, cb, w], F16, tag="iR")
                        iS = s_p.tile([128, cb, w], F16, tag="iS")
                        it = s_p.tile([128, cb, w], F16, tag="it")
                        # DVE may read only one PSUM operand per op: ACT
                        # drains one plane of each +/- pair to fp16 SBUF.
                        m5s = s_p.tile([128, cb, w], F16, tag="m5s")
                        m3s = s_p.tile([128, cb, w], F16, tag="m3s")
                        nc.scalar.copy(m5s[:], ms[5][:])
                        nc.scalar.copy(m3s[:], ms[3][:])
                        # P=M1+M2=p4-p5  Q=M1-M2=p4+p5  R=M3+M4  S=M3-M4
                        nc.vector.tensor_sub(iP[:], ms[4][:], m5s[:])
                        nc.vector.tensor_add(iQ[:], ms[4][:], m5s[:])
                        nc.vector.tensor_add(iR[:], ms[2][:], m3s[:])
                        nc.vector.tensor_sub(iS[:], ms[2][:], m3s[:])
                        # o0 = M0 + P + R
                        nc.vector.tensor_add(it[:], ms[0][:], iR[:])
                        nc.vector.tensor_add(obv[:, 0], it[:], iP[:])
                        # o1 = Q + 2S
                        nc.vector.tensor_scalar_mul(it[:], iS[:], 2.0)
                        nc.vector.tensor_add(obv[:, 1], iQ[:], it[:])
                        # o2 = P + 4R
                        nc.vector.tensor_scalar_mul(it[:], iR[:], 4.0)
                        nc.vector.tensor_add(obv[:, 2], iP[:], it[:])
                        # o3 = Q + 8S + M5
                        nc.vector.tensor_scalar_mul(it[:], iS[:], 8.0)
                        nc.vector.tensor_add(it[:], iQ[:], it[:])
                        nc.vector.tensor_add(obv[:, 3], it[:], ms[1][:])
                        nc.sync.dma_start(
                            out=o_d[n, oc * 128:(oc + 1) * 128,
                                    ch * M * cb:(ch + 1) * M * cb, :],
                            in_=ob[:])

    nc.compile()
    return nc


_CACHE = {}


def _get_program():
    if "nc" not in _CACHE:
        _CACHE["nc"] = build_program()
    return _CACHE["nc"]


def kernel(x, weight, bias):
    x = np.ascontiguousarray(x, dtype=np.float32)
    weight = np.ascontiguousarray(weight, dtype=np.float32)
    bias = np.ascontiguousarray(bias, dtype=np.float32)
    nc = _get_program()
    in_maps = [
        {"x": x[c * BPC:(c + 1) * BPC], "weight": weight, "bias": bias}
        for c in range(N_CORES)
    ]
    r = run_bass_kernel_spmd(nc, in_maps, list(range(N_CORES)))
    return np.concatenate(
        [r.results[c]["out"].astype(np.float32) for c in range(N_CORES)],
        axis=0)
